# revision 39
# baseline (speedup 1.0000x reference)
"""Distributed attention kernel for 8 TRN2 NeuronCores (v8: p2p gather).

Problem: x[8192,1024] @ {W_q,W_k,W_v}[1024,128] -> softmax(QK^T/sqrt(128)) @ V.

Sharding: x row-sharded (1024 rows/core), weights replicated. Each core
computes K^T_loc/V_loc from its shard and exchanges shards p2p with
remote_dma_broadcast (direct SBUF->SBUF), then attends its own 1024 Q rows
against the full K/V. ~273-290us vs the v3 collective baseline's 300-325us.

v8 design notes:
  - XOR slot layout: core r stores core s's K/V shard at slot d = s ^ r.
    Every remote_dma_broadcast then uses a compile-time dest offset (slot d
    == relative tpb d), and the d&4 <-> slot&4 D2D constraint holds for
    free. Softmax + A@V are invariant under a consistent k-permutation, so
    no un-permutation is ever needed.
  - p2p replaces the RDH AllGather via shared DRAM. HARD CONSTRAINT learned
    on HW: cross-core traffic sent before the NRT sync barrier (~T_B=68us)
    is silently lost (receiver hangs); the earliest bass-visible proof of
    T_B is the kernel-entry prelude AllGather's then_inc (~T_B+20). The
    send triggers are gated on it; descgen preps run early ungated. Shards
    land in peer SBUF at ~97us (7x256KB over 2 DMA engines each, ~150GB/s
    aggregate), vs ~94-100us for the v3 collective+DRAM-gather chain - the
    win is the removed gather hop + progressive consumption below.
  - progressive stats: stats matmuls are gated per-slot by arrival
    semaphores attached POST-SCHEDULE (the Tile scheduler's single-core
    CoreSim cannot see peer increments and would report deadlock).
  - warm-matmul bridge: ~170 junk matmuls on the 4-bank ps_st ring (so
    they pipeline back-to-back) keep the PE HAM activity monitor armed
    across the send-gate window; explicit add_dep_helper edges force the
    arrival-gated stats matmuls behind them, else the scheduler interleaves
    them and the PE queue stalls at the first gated matmul (clock drops to
    4/8 and stays down through stats, +40us; measured v6/v7).
  - stats max-reduce: DVE InstTensorReduce has NO 2x/4x fast modes
    (~675ns/[128,512] always), so the full max-scan of S is ~86us of DVE
    per core and paces the stats phase; 2/3 of units route via an ACT
    fp16 copy to spread latency. GpSimd cannot free-axis reduce (axis C
    only). Splitting stats across main_0 (v3-style) retested WORSE here
    (clock dips at phase transitions; 311us).
  - main chunks: per chunk 64 shift (PSUM preload of -m via identity
    matmul, full 128-row activity) + 64 S^T + 64 AV + 8 den matmuls (den
    via oct-summed aT: three DVE bf16 2x-mode add layers); exp on ACT
    (~686ns/tile, ~44us/chunk) and PE (~50us/chunk) are co-pacers.
  - a clock-hold warm matmul follows each arrival-gated stats unit (dep
    edge keeps it behind the unit): the stats phase alone has ~32% PE duty
    and the HAM drops the core clock, throttling the pacing DVE with it.
  - run-to-run variance (~273-290us for the same NEFF) is dominated by a
    separate 13/16 power/thermal throttle (ham type 31) that can engage
    for the whole mid-kernel even at 166% PE duty - distinct from the
    activity-based 4/8 drop (type 1), and not controllable in-kernel:
    stationary-load sharing (kv-outer) was evaluated and skipped since
    measured matmul spacing (~242ns vs 213 theoretical) shows LDWEIGHTS
    already hides behind the previous matmul's moving phase.

Numerics: logits have std ~1024 (randn inputs); softmax is near-one-hot so
the Q/K/S path needs |logit err| << 1: fp16 (11-bit mantissa) gives ~0.15.
The shift value's accuracy is irrelevant (cancels in the normalization).
rel_err 1.335e-02 (threshold 2e-2), identical to the v3 baseline.
"""

import os
import sys

import numpy as np

os.environ.setdefault("MYCRO_LOCAL_CACHE", "1")

try:
    import concourse  # noqa: F401
except ImportError:  # pragma: no cover - path fallback for fresh dirs
    for _p in ("/opt/trn_rl_repo", "/root/.axon_site/_ro/trn_rl_repo"):
        if os.path.isdir(_p):
            sys.path.insert(0, _p)
    import concourse  # noqa: F401

import concourse.bass as bass
import concourse.mybir as mybir
import concourse.tile as tile
from concourse import bacc
from concourse.bass_utils import run_bass_kernel_spmd
from concourse.masks import make_identity
from concourse.tile_rust import add_dep_helper

F32 = mybir.dt.float32
F32R = mybir.dt.float32r
F16 = mybir.dt.float16
BF16 = mybir.dt.bfloat16

N_CORES = 8
P = 128
NTOK = 8192
DIN = 1024
DQK = 128
DV = 128
NLOC = NTOK // N_CORES  # 1024 rows per core
TQ = NLOC // P  # 8 q tiles per core
TD = DIN // P  # 8 d_in tiles
NKV = NTOK // P  # 64 kv tiles
QC = 512  # q-chunk width for the main phase
NQC = NLOC // QC  # 2 q-chunks
SCH = 512  # stats chunk width (one fp32 psum bank)
NSCH = NTOK // SCH  # 16 stats chunks per q-tile
SCALE = 1.0 / float(np.sqrt(DQK))
PIPE = 3  # software pipeline depth (kv tiles) between S^T matmul and AV


def build_nc():
    nc = bacc.Bacc(
        "TRN2",
        target_bir_lowering=False,
        debug=False,
        enable_asserts=False,
        num_devices=N_CORES,
        monotonic_sem_count=16,
    )

    x_d = nc.dram_tensor("x", [NLOC, DIN], F32, kind="ExternalInput").ap()
    wq_d = nc.dram_tensor("W_q", [DIN, DQK], F32, kind="ExternalInput").ap()
    wk_d = nc.dram_tensor("W_k", [DIN, DQK], F32, kind="ExternalInput").ap()
    wv_d = nc.dram_tensor("W_v", [DIN, DV], F32, kind="ExternalInput").ap()
    out_d = nc.dram_tensor("out", [NLOC, DV], F32, kind="ExternalOutput").ap()

    groups = [list(range(N_CORES))]

    # p2p arrival semaphores: slot d (=src^self) of K -> sem d-1, of V ->
    # sem 6+d. Each single-dest broadcast bumps the dest's sem by 16//8 = 2.
    sem_k = [nc.monotonic_semaphore(d - 1).sem() for d in range(1, N_CORES)]
    sem_v = [nc.monotonic_semaphore(6 + d).sem() for d in range(1, N_CORES)]
    sem_spare = nc.monotonic_semaphore(14).sem()  # local_sem sink, not waited

    # Cross-core waits must be attached AFTER Tile's scheduling pass: the
    # scheduler's single-core CoreSim cannot see peer increments and would
    # report a deadlock. We collect (inst, sem, val) here and attach the
    # sem-ge conditions post-schedule; compile's generate_event_semaphores
    # legalizes wait counts and move_matmul_waits_to_ldweights relocates
    # matmul waits.
    pend_waits = []

    with tile.TileContext(nc) as tc:
        with (
            tc.tile_pool(name="consts", bufs=1) as consts,
            tc.tile_pool(name="persist", bufs=1) as persist,
        ):
            ident_f32 = consts.tile([P, P], F32)
            make_identity(nc, ident_f32)
            ident_h = consts.tile([P, P], F16)
            nc.vector.tensor_copy(out=ident_h, in_=ident_f32)
            ones_f = consts.tile([1, P], F32)
            nc.vector.memset(ones_f, 1.0)
            ones_col_bf = consts.tile([P, 1], BF16)
            nc.vector.memset(ones_col_bf, 1.0)

            # Persistent SBUF tensors. Slot 0 of kT_h / vf is the local
            # shard (written directly by the projections); slots 1..7 are
            # remote-written by the peers' broadcasts.
            qT_h = persist.tile([P, NLOC], F16)  # Q^T, pre-scaled, fp16
            kT_h = persist.tile([P, NTOK], F16)  # K^T by slot, fp16
            vf = persist.tile([P, NKV, P], BF16)  # V tiles by slot
            negm_row = persist.tile([1, NLOC], F16)
            nb = persist.tile([P, NQC, QC], F16)  # -m broadcast per chunk
            mx_all = persist.tile([P, TQ, NSCH], F32)

            # ---------------- projections + p2p exchange ----------------
            with (
                tc.tile_pool(name="proj_sb", bufs=1) as proj_sb,
                tc.tile_pool(name="ps_xt", bufs=2, space="PSUM") as ps_xt_pool,
                tc.tile_pool(name="ps_mm", bufs=2, space="PSUM") as ps_mm_pool,
                tc.tile_pool(name="ps_v", bufs=2, space="PSUM") as ps_v_pool,
            ):
                xa = proj_sb.tile([P, TQ, DIN], F32)
                xT_r = proj_sb.tile([P, TD, NLOC], F32R)
                xT_bf = proj_sb.tile([P, TD, NLOC], BF16)
                wq = proj_sb.tile([P, TD, DQK], F32)
                wk = proj_sb.tile([P, TD, DQK], F32)
                wv = proj_sb.tile([P, TD, DV], F32)
                wq_r = proj_sb.tile([P, TD, DQK], F32R)
                wk_r = proj_sb.tile([P, TD, DQK], F32R)
                wv_bf = proj_sb.tile([P, TD, DV], BF16)

                with nc.named_scope("load"):
                    # W_k first (it gates the K projection -> broadcast) and
                    # per-tile contiguous DMAs.
                    for di in range(TD):
                        nc.sync.dma_start(
                            out=wk[:, di, :], in_=wk_d[di * P : (di + 1) * P, :]
                        )
                    for tj in range(TQ):
                        nc.sync.dma_start(
                            out=xa[:, tj, :], in_=x_d[tj * P : (tj + 1) * P, :]
                        )
                    for di in range(TD):
                        nc.sync.dma_start(
                            out=wv[:, di, :], in_=wv_d[di * P : (di + 1) * P, :]
                        )
                    for di in range(TD):
                        nc.sync.dma_start(
                            out=wq[:, di, :], in_=wq_d[di * P : (di + 1) * P, :]
                        )
                    nc.vector.tensor_copy(out=wk_r, in_=wk)

                # x^T (PE transposes), evacuations split across DVE/ACT.
                for tg in range(2):
                    with nc.named_scope(f"xT_{tg}"):
                        for di in range(TD):
                            ps_xt = ps_xt_pool.tile([P, 4 * P], F32, tag="ps_xt")
                            for j in range(4):
                                tj = tg * 4 + j
                                nc.tensor.transpose(
                                    ps_xt[:, j * P : (j + 1) * P],
                                    xa[:, tj, di * P : (di + 1) * P],
                                    ident_f32,
                                )
                            sl = slice(tg * 4 * P, (tg + 1) * 4 * P)
                            if di % 2 == 0:
                                nc.vector.tensor_copy(
                                    out=xT_r[:, di, sl], in_=ps_xt
                                )
                            else:
                                nc.scalar.copy(out=xT_r[:, di, sl], in_=ps_xt)
                    with nc.named_scope(f"kT_proj_{tg}"):
                        ps_k = ps_mm_pool.tile([P, 512], F32, tag="ps_mm")
                        for di in range(TD):
                            nc.tensor.matmul(
                                ps_k,
                                wk_r[:, di, :],
                                xT_r[:, di, tg * 512 : (tg + 1) * 512],
                                start=(di == 0),
                                stop=(di == TD - 1),
                            )
                        nc.vector.tensor_copy(
                            out=kT_h[:, tg * 512 : (tg + 1) * 512], in_=ps_k
                        )

                # p2p K broadcast: slot d == relative tpb d (XOR routing).
                # Cross-core traffic is only safe after the NRT sync
                # barrier (T_B, ~68us); the earliest bass-visible proof is
                # the prelude AllGather's then_inc on the kernel-entry
                # barrier sem (~T_B+20us). Gate the triggers on it (preps
                # are descgen only, no traffic). Sending earlier loses
                # packets and hangs the receivers' arrival waits.
                bsem = nc._bir_kernel_barrier_sem
                binc = nc.bir_kernel_barrier_sem_inc
                with nc.named_scope("bc_k"):
                    for d in range(1, N_CORES):
                        rdests = [None] * N_CORES
                        rdests[d] = (0, d)
                        nc.gpsimd.remote_dma_broadcast(
                            out_ap=kT_h[:, d * NLOC : (d + 1) * NLOC],
                            in_ap=kT_h[:, 0:NLOC],
                            remote_sem=sem_k[d - 1],
                            local_sem=sem_spare,
                            rdests=rdests,
                        )
                    trig = nc.gpsimd.trigger_dma(count=None)
                    pend_waits.append((trig, bsem, binc))

                with nc.named_scope("q_proj"):
                    nc.vector.tensor_copy(out=wq_r, in_=wq)
                    for h in range(NLOC // 512):
                        ps_q = ps_mm_pool.tile([P, 512], F32, tag="ps_mm")
                        for di in range(TD):
                            nc.tensor.matmul(
                                ps_q,
                                wq_r[:, di, :],
                                xT_r[:, di, h * 512 : (h + 1) * 512],
                                start=(di == 0),
                                stop=(di == TD - 1),
                            )
                        nc.vector.tensor_scalar_mul(
                            qT_h[:, h * 512 : (h + 1) * 512], ps_q, SCALE
                        )

                with nc.named_scope("v_proj"):
                    nc.vector.tensor_copy(out=wv_bf, in_=wv)
                    for di in range(TD):
                        nc.scalar.copy(
                            out=xT_bf[:, di, :], in_=xT_r[:, di, :].bitcast(F32)
                        )
                    for tj in range(TQ):
                        ps_v = ps_v_pool.tile([P, DV], F32, tag="ps_v")
                        for di in range(TD):
                            nc.tensor.matmul(
                                ps_v,
                                xT_bf[:, di, tj * P : (tj + 1) * P],
                                wv_bf[:, di, :],
                                start=(di == 0),
                                stop=(di == TD - 1),
                            )
                        nc.vector.tensor_copy(out=vf[:, tj, :], in_=ps_v)

                with nc.named_scope("bc_v"):
                    for d in range(1, N_CORES):
                        rdests = [None] * N_CORES
                        rdests[d] = (0, d)
                        nc.gpsimd.remote_dma_broadcast(
                            out_ap=vf[:, d * TQ : (d + 1) * TQ, :],
                            in_ap=vf[:, 0:TQ, :],
                            remote_sem=sem_v[d - 1],
                            local_sem=sem_spare,
                            rdests=rdests,
                        )
                    trig = nc.gpsimd.trigger_dma(count=None)
                    pend_waits.append((trig, bsem, binc))

            # ---------------- attention ----------------
            with (
                tc.tile_pool(name="attn_sb", bufs=4) as attn_sb,
                tc.tile_pool(name="stat_sb", bufs=2) as stat_sb,
                tc.tile_pool(name="ps_st", bufs=4, space="PSUM") as ps_st_pool,
                tc.tile_pool(name="ps_stat", bufs=2, space="PSUM") as ps_stat_pool,
                tc.tile_pool(name="ps_od", bufs=1, space="PSUM") as ps_od_pool,
            ):
                # Multi-use PSUM bank: den accumulators on rows 0 (chunk 0)
                # and 32 (chunk 1); den-transpose scratch (only touched after
                # the den row has been read out).
                ps_misc = ps_od_pool.tile([P, QC], F32, tag="ps_misc", bufs=1)

                route_ctr = [0]

                def stats_unit(qt, ch, kwait):
                    """One stats chunk: fp16 matmul + max-reduce."""
                    ps_stat = ps_st_pool.tile([P, SCH], F32, tag="ps_st")
                    mm = nc.tensor.matmul(
                        ps_stat,
                        qT_h[:, qt * P : (qt + 1) * P],
                        kT_h[:, ch * SCH : (ch + 1) * SCH],
                        start=True,
                        stop=True,
                    )
                    if kwait is not None:
                        pend_waits.append((mm, kwait[0], kwait[1]))
                        if warm_last[0] is not None:
                            add_dep_helper(
                                mm.ins,
                                warm_last[0].ins,
                                reason="gated stats after warm bridge",
                            )
                    route_ctr[0] += 1
                    if route_ctr[0] % 3 == 0:
                        nc.vector.reduce_max(
                            mx_all[:, qt, ch : ch + 1],
                            ps_stat,
                            axis=mybir.AxisListType.X,
                        )
                    else:
                        sh = stat_sb.tile([P, SCH], F16, tag="stat_h", bufs=3)
                        nc.scalar.copy(out=sh, in_=ps_stat)
                        nc.vector.reduce_max(
                            mx_all[:, qt, ch : ch + 1],
                            sh,
                            axis=mybir.AxisListType.X,
                        )
                    return mm

                def stats_combine(qt):
                    """Combine chunk maxes -> -m_hat -> negm_row slice."""
                    m1 = stat_sb.tile([P, 1], F32, tag="m1")
                    negm = stat_sb.tile([P, 1], F32, tag="negm")
                    nc.vector.reduce_max(
                        m1, mx_all[:, qt, :], axis=mybir.AxisListType.X
                    )
                    nc.vector.tensor_scalar_mul(negm, m1, -1.0)
                    ps_nm = ps_stat_pool.tile([1, P], F32, tag="ps_stat")
                    nc.tensor.transpose(ps_nm, negm, ident_f32)
                    nc.vector.tensor_copy(
                        out=negm_row[0:1, qt * P : (qt + 1) * P], in_=ps_nm
                    )

                def warm_mm(col):
                    """Junk matmul into the (otherwise idle) ps_o bank:
                    keeps the PE HAM activity monitor armed across the
                    barrier-gated wait for the remote K shards, so the
                    remote-slot stats and main run at full clock."""
                    ps_w = ps_st_pool.tile([P, QC], F32, tag="ps_st")
                    return nc.tensor.matmul(
                        ps_w,
                        ident_h,
                        kT_h[:, col : col + QC],
                        start=True,
                        stop=True,
                    )

                # Stats, slot-major: local shard first, then remote shards
                # as they arrive (gated by the per-slot arrival semaphores).
                # The warm bridge covers local-work-done (~55us) to
                # shard-arrival (~my send trigger + transfer, ~95us).
                warm_last = [None]
                with nc.named_scope("stats"):
                    for d in range(N_CORES):
                        kwait = (sem_k[d - 1], 2) if d > 0 else None
                        if d == 1:
                            with nc.named_scope("warm"):
                                for w in range(170):
                                    warm_last[0] = warm_mm((w % 2) * QC)
                        for qt in range(TQ):
                            for j in range(2):
                                smm = stats_unit(qt, 2 * d + j, kwait)
                                if d > 0:
                                    ps_w = ps_od_pool.tile(
                                        [P, QC], F32, tag="ps_o", bufs=1
                                    )
                                    wmm = nc.tensor.matmul(
                                        ps_w,
                                        ident_h,
                                        kT_h[:, j * QC : (j + 1) * QC],
                                        start=True,
                                        stop=True,
                                    )
                                    add_dep_helper(
                                        wmm.ins,
                                        smm.ins,
                                        reason="clock-hold warm after stats",
                                    )
                            if d == N_CORES - 1:
                                stats_combine(qt)

                def shift_prologue(qc, ps_st):
                    """PSUM <- ident^T @ nb (full-activity shift)."""
                    nc.tensor.matmul(
                        ps_st, ident_h, nb[:, qc, :], start=True, stop=False
                    )

                def st_accum(qc, kv, ps_st):
                    qs = qc * QC
                    nc.tensor.matmul(
                        ps_st,
                        kT_h[:, kv * P : (kv + 1) * P],
                        qT_h[:, qs : qs + QC],
                        start=False,
                        stop=True,
                    )

                def out_phase(qc, ps_o, ps_den):
                    """Evacuate O^T + den for chunk qc: transpose, scale, DMA."""
                    qs = qc * QC
                    den_row = stat_sb.tile([1, QC], F32, tag="den_row")
                    nc.vector.tensor_copy(out=den_row, in_=ps_den)
                    ps_rd = ps_misc[:, 0 : QC // P]
                    for j in range(QC // P):
                        nc.tensor.transpose(
                            ps_rd[:, j : j + 1],
                            den_row[0:1, j * P : (j + 1) * P],
                            ones_f[0:1, 0:1],
                        )
                    # reciprocal AFTER transposing to [128, 4]: 128 DVE lanes
                    # instead of one (a [1,512] reciprocal costs 3.3us serial)
                    den_col = stat_sb.tile([P, QC // P], F32, tag="den_col")
                    nc.vector.tensor_copy(out=den_col, in_=ps_rd)
                    rden_col = stat_sb.tile([P, QC // P], F32, tag="rden_col")
                    nc.vector.reciprocal(rden_col, den_col)

                    oT_sb = stat_sb.tile([P, QC], F32, tag="oT_sb")
                    nc.vector.tensor_copy(out=oT_sb, in_=ps_o)
                    o_nat = stat_sb.tile([P, QC // P, DV], F32, tag="o_nat")
                    ps_on = ps_st_pool.tile([P, QC], F32, tag="ps_st")
                    for j in range(QC // P):
                        nc.tensor.transpose(
                            ps_on[:, j * P : (j + 1) * P],
                            oT_sb[:, j * P : (j + 1) * P],
                            ident_f32,
                        )
                    for j in range(QC // P):
                        nc.vector.tensor_scalar_mul(
                            o_nat[:, j, :],
                            ps_on[:, j * P : (j + 1) * P],
                            rden_col[:, j : j + 1],
                        )
                    nc.sync.dma_start(
                        out=out_d[qs : qs + QC, :].rearrange(
                            "(t p) d -> p t d", p=P
                        ),
                        in_=o_nat,
                    )

                # main chunks (stats complete before main_0)
                pending_out = []
                for qc in range(NQC):
                    with nc.named_scope(f"main_{qc}"):
                        nc.gpsimd.partition_broadcast(
                            nb[:, qc, :], negm_row[0:1, qc * QC : (qc + 1) * QC]
                        )
                        ps_o = ps_od_pool.tile([P, QC], F32, tag="ps_o", bufs=1)
                        ps_den = ps_misc[qc * 32 : qc * 32 + 1, :]
                        aT_tiles = {}
                        aTs_tiles = {}
                        aTq_tiles = {}
                        aTo_tiles = {}
                        ps_tiles = {}
                        for kv in range(NKV + PIPE):
                            if kv < NKV:
                                if kv % 2 == 0:
                                    # both shift prologues back-to-back: the
                                    # identity stationary loads only once
                                    ps_tiles[kv] = ps_st_pool.tile(
                                        [P, QC], F32, tag="ps_st", name="ps_a"
                                    )
                                    ps_tiles[kv + 1] = ps_st_pool.tile(
                                        [P, QC], F32, tag="ps_st", name="ps_b"
                                    )
                                    shift_prologue(qc, ps_tiles[kv])
                                    shift_prologue(qc, ps_tiles[kv + 1])
                                ps_st = ps_tiles.pop(kv)
                                st_accum(qc, kv, ps_st)
                                if qc == 1 and kv == 1 and pending_out:
                                    with nc.named_scope("out_0"):
                                        out_phase(*pending_out.pop())
                                aT = attn_sb.tile([P, QC], BF16, tag="aT", bufs=6)
                                nc.scalar.activation(
                                    aT, ps_st, mybir.ActivationFunctionType.Exp
                                )
                                aT_tiles[kv] = aT
                                if kv % 2 == 1:
                                    # pair-sum then quad-sum in bf16 (DVE 4x
                                    # mode) to quarter the den matmuls
                                    aTs = attn_sb.tile(
                                        [P, QC], BF16, tag="aTs", bufs=3
                                    )
                                    nc.vector.tensor_tensor(
                                        aTs,
                                        aT_tiles[kv - 1],
                                        aT_tiles[kv],
                                        mybir.AluOpType.add,
                                    )
                                    aTs_tiles[kv // 2] = aTs
                                if kv % 4 == 3:
                                    aTq = attn_sb.tile(
                                        [P, QC], BF16, tag="aTq", bufs=3
                                    )
                                    nc.vector.tensor_tensor(
                                        aTq,
                                        aTs_tiles.pop(kv // 2 - 1),
                                        aTs_tiles.pop(kv // 2),
                                        mybir.AluOpType.add,
                                    )
                                    aTq_tiles[kv // 4] = aTq
                                if kv % 8 == 7:
                                    aTo = attn_sb.tile(
                                        [P, QC], BF16, tag="aTo", bufs=3
                                    )
                                    nc.vector.tensor_tensor(
                                        aTo,
                                        aTq_tiles.pop(kv // 4 - 1),
                                        aTq_tiles.pop(kv // 4),
                                        mybir.AluOpType.add,
                                    )
                                    aTo_tiles[kv // 8] = aTo
                            k2 = kv - PIPE
                            if k2 >= 0:
                                av = nc.tensor.matmul(
                                    ps_o,
                                    vf[:, k2, :],
                                    aT_tiles[k2],
                                    start=(k2 == 0),
                                    stop=(k2 == NKV - 1),
                                )
                                slot = k2 // TQ
                                if slot > 0:
                                    pend_waits.append((av, sem_v[slot - 1], 2))
                                if k2 % 8 == 7:
                                    qr = k2 // 8
                                    nc.tensor.matmul(
                                        ps_den,
                                        ones_col_bf,
                                        aTo_tiles[qr],
                                        start=(qr == 0),
                                        stop=(qr == NKV // 8 - 1),
                                    )
                                    del aTo_tiles[qr]
                                del aT_tiles[k2]
                    if qc == 0:
                        pending_out.append((qc, ps_o, ps_den))
                    else:
                        with nc.named_scope(f"out_{qc}"):
                            out_phase(qc, ps_o, ps_den)

    # Attach the cross-core sem-ge waits now that scheduling is done.
    # The prelude AllGather (nothing waits on it) keeps one collective in
    # the NEFF so the runtime initializes cross-core comms.
    nc._bir_kernel_barrier_sem_replica_groups.extend(set(g) for g in groups)
    for bi, sem, val in pend_waits:
        # check=False: scheduler-assigned waits may already be present;
        # compile's generate_event_semaphores legalizes the wait count.
        bi.wait_op(sem, val, "sem-ge", check=False)

    nc.compile()
    return nc


_NC_CACHE = None


def _get_nc():
    global _NC_CACHE
    if _NC_CACHE is None:
        _NC_CACHE = build_nc()
    return _NC_CACHE


def run(inputs, trace=False, **kw):
    """Run the SPMD kernel; returns BassKernelResults."""
    nc = _get_nc()
    x = np.asarray(inputs["x"], dtype=np.float32)
    wq = np.asarray(inputs["W_q"], dtype=np.float32)
    wk = np.asarray(inputs["W_k"], dtype=np.float32)
    wv = np.asarray(inputs["W_v"], dtype=np.float32)
    in_maps = [
        {
            "x": np.ascontiguousarray(x[c * NLOC : (c + 1) * NLOC]),
            "W_q": wq,
            "W_k": wk,
            "W_v": wv,
        }
        for c in range(N_CORES)
    ]
    return run_bass_kernel_spmd(
        nc, in_maps, core_ids=list(range(N_CORES)), trace=trace, **kw
    )


def kernel(**inputs):
    res = run(inputs, trace=False)
    return np.concatenate([res.results[c]["out"] for c in range(N_CORES)], axis=0)


# revision 40
# speedup vs baseline: 1.0189x; 1.0189x over previous
"""Distributed attention kernel for 8 TRN2 NeuronCores (v8: p2p gather).

Problem: x[8192,1024] @ {W_q,W_k,W_v}[1024,128] -> softmax(QK^T/sqrt(128)) @ V.

Sharding: x row-sharded (1024 rows/core), weights replicated. Each core
computes K^T_loc/V_loc from its shard and exchanges shards p2p with
remote_dma_broadcast (direct SBUF->SBUF), then attends its own 1024 Q rows
against the full K/V. ~273-290us vs the v3 collective baseline's 300-325us.

v8 design notes:
  - XOR slot layout: core r stores core s's K/V shard at slot d = s ^ r.
    Every remote_dma_broadcast then uses a compile-time dest offset (slot d
    == relative tpb d), and the d&4 <-> slot&4 D2D constraint holds for
    free. Softmax + A@V are invariant under a consistent k-permutation, so
    no un-permutation is ever needed.
  - p2p replaces the RDH AllGather via shared DRAM. HARD CONSTRAINT learned
    on HW: cross-core traffic sent before the NRT sync barrier (~T_B=68us)
    is silently lost (receiver hangs); the earliest bass-visible proof of
    T_B is the kernel-entry prelude AllGather's then_inc (~T_B+20). The
    send triggers are gated on it; descgen preps run early ungated. Shards
    land in peer SBUF at ~97us (7x256KB over 2 DMA engines each, ~150GB/s
    aggregate), vs ~94-100us for the v3 collective+DRAM-gather chain - the
    win is the removed gather hop + progressive consumption below.
  - progressive stats: stats matmuls are gated per-slot by arrival
    semaphores attached POST-SCHEDULE (the Tile scheduler's single-core
    CoreSim cannot see peer increments and would report deadlock).
  - warm-matmul bridge: ~170 junk matmuls on the 4-bank ps_st ring (so
    they pipeline back-to-back) keep the PE HAM activity monitor armed
    across the send-gate window; explicit add_dep_helper edges force the
    arrival-gated stats matmuls behind them, else the scheduler interleaves
    them and the PE queue stalls at the first gated matmul (clock drops to
    4/8 and stays down through stats, +40us; measured v6/v7).
  - stats max-reduce: DVE InstTensorReduce has NO 2x/4x fast modes
    (~675ns/[128,512] always), so the full max-scan of S is ~86us of DVE
    per core and paces the stats phase; 2/3 of units route via an ACT
    fp16 copy to spread latency. GpSimd cannot free-axis reduce (axis C
    only). Splitting stats across main_0 (v3-style) retested WORSE here
    (clock dips at phase transitions; 311us).
  - main chunks: per chunk 64 shift (PSUM preload of -m via identity
    matmul, full 128-row activity) + 64 S^T + 64 AV + 8 den matmuls (den
    via oct-summed aT: three DVE bf16 2x-mode add layers); exp on ACT
    (~686ns/tile, ~44us/chunk) and PE (~50us/chunk) are co-pacers.
  - a clock-hold warm matmul follows each arrival-gated stats unit (dep
    edge keeps it behind the unit): the stats phase alone has ~32% PE duty
    and the HAM drops the core clock, throttling the pacing DVE with it.
  - run-to-run variance (~273-290us for the same NEFF) is dominated by a
    separate 13/16 power/thermal throttle (ham type 31) that can engage
    for the whole mid-kernel even at 166% PE duty - distinct from the
    activity-based 4/8 drop (type 1), and not controllable in-kernel:
    stationary-load sharing (kv-outer) was evaluated and skipped since
    measured matmul spacing (~242ns vs 213 theoretical) shows LDWEIGHTS
    already hides behind the previous matmul's moving phase.

Numerics: logits have std ~1024 (randn inputs); softmax is near-one-hot so
the Q/K/S path needs |logit err| << 1: fp16 (11-bit mantissa) gives ~0.15.
The shift value's accuracy is irrelevant (cancels in the normalization).
rel_err 1.335e-02 (threshold 2e-2), identical to the v3 baseline.
"""

import os
import sys

import numpy as np

os.environ.setdefault("MYCRO_LOCAL_CACHE", "1")

try:
    import concourse  # noqa: F401
except ImportError:  # pragma: no cover - path fallback for fresh dirs
    for _p in ("/opt/trn_rl_repo", "/root/.axon_site/_ro/trn_rl_repo"):
        if os.path.isdir(_p):
            sys.path.insert(0, _p)
    import concourse  # noqa: F401

import concourse.bass as bass
import concourse.mybir as mybir
import concourse.tile as tile
from concourse import bacc
from concourse.bass_utils import run_bass_kernel_spmd
from concourse.masks import make_identity
from concourse.tile_rust import add_dep_helper

F32 = mybir.dt.float32
F32R = mybir.dt.float32r
F16 = mybir.dt.float16
BF16 = mybir.dt.bfloat16

N_CORES = 8
P = 128
NTOK = 8192
DIN = 1024
DQK = 128
DV = 128
NLOC = NTOK // N_CORES  # 1024 rows per core
TQ = NLOC // P  # 8 q tiles per core
TD = DIN // P  # 8 d_in tiles
NKV = NTOK // P  # 64 kv tiles
QC = 512  # q-chunk width for the main phase
NQC = NLOC // QC  # 2 q-chunks
SCH = 512  # stats chunk width (one fp32 psum bank)
NSCH = NTOK // SCH  # 16 stats chunks per q-tile
SCALE = 1.0 / float(np.sqrt(DQK))
PIPE = 3  # software pipeline depth (kv tiles) between S^T matmul and AV


def build_nc():
    nc = bacc.Bacc(
        "TRN2",
        target_bir_lowering=False,
        debug=False,
        enable_asserts=False,
        num_devices=N_CORES,
        monotonic_sem_count=16,
    )

    x_d = nc.dram_tensor("x", [NLOC, DIN], F32, kind="ExternalInput").ap()
    wq_d = nc.dram_tensor("W_q", [DIN, DQK], F32, kind="ExternalInput").ap()
    wk_d = nc.dram_tensor("W_k", [DIN, DQK], F32, kind="ExternalInput").ap()
    wv_d = nc.dram_tensor("W_v", [DIN, DV], F32, kind="ExternalInput").ap()
    out_d = nc.dram_tensor("out", [NLOC, DV], F32, kind="ExternalOutput").ap()

    groups = [list(range(N_CORES))]

    # p2p arrival semaphores: slot d (=src^self) of K -> sem d-1, of V ->
    # sem 6+d. Each single-dest broadcast bumps the dest's sem by 16//8 = 2.
    sem_k = [nc.monotonic_semaphore(d - 1).sem() for d in range(1, N_CORES)]
    sem_v = [nc.monotonic_semaphore(6 + d).sem() for d in range(1, N_CORES)]
    sem_spare = nc.monotonic_semaphore(14).sem()  # local_sem sink, not waited

    # Cross-core waits must be attached AFTER Tile's scheduling pass: the
    # scheduler's single-core CoreSim cannot see peer increments and would
    # report a deadlock. We collect (inst, sem, val) here and attach the
    # sem-ge conditions post-schedule; compile's generate_event_semaphores
    # legalizes wait counts and move_matmul_waits_to_ldweights relocates
    # matmul waits.
    pend_waits = []

    with tile.TileContext(nc) as tc:
        with (
            tc.tile_pool(name="consts", bufs=1) as consts,
            tc.tile_pool(name="persist", bufs=1) as persist,
        ):
            ident_f32 = consts.tile([P, P], F32)
            make_identity(nc, ident_f32)
            ident_h = consts.tile([P, P], F16)
            nc.vector.tensor_copy(out=ident_h, in_=ident_f32)
            ones_f = consts.tile([1, P], F32)
            nc.vector.memset(ones_f, 1.0)
            ones_col_bf = consts.tile([P, 1], BF16)
            nc.vector.memset(ones_col_bf, 1.0)

            # Persistent SBUF tensors. Slot 0 of kT_h / vf is the local
            # shard (written directly by the projections); slots 1..7 are
            # remote-written by the peers' broadcasts.
            qT_h = persist.tile([P, NLOC], F16)  # Q^T, pre-scaled, fp16
            kT_h = persist.tile([P, NTOK], F16)  # K^T by slot, fp16
            vf = persist.tile([P, NKV, P], BF16)  # V tiles by slot
            negm_row = persist.tile([1, NLOC], F16)
            nb = persist.tile([P, NQC, QC], F16)  # -m broadcast per chunk
            mx_all = persist.tile([P, TQ, NSCH], F32)

            # ---------------- projections + p2p exchange ----------------
            with (
                tc.tile_pool(name="proj_sb", bufs=1) as proj_sb,
                tc.tile_pool(name="ps_xt", bufs=2, space="PSUM") as ps_xt_pool,
                tc.tile_pool(name="ps_mm", bufs=2, space="PSUM") as ps_mm_pool,
                tc.tile_pool(name="ps_v", bufs=2, space="PSUM") as ps_v_pool,
            ):
                xa = proj_sb.tile([P, TQ, DIN], F32)
                xT_r = proj_sb.tile([P, TD, NLOC], F32R)
                xT_bf = proj_sb.tile([P, TD, NLOC], BF16)
                wq = proj_sb.tile([P, TD, DQK], F32)
                wk = proj_sb.tile([P, TD, DQK], F32)
                wv = proj_sb.tile([P, TD, DV], F32)
                wq_r = proj_sb.tile([P, TD, DQK], F32R)
                wk_r = proj_sb.tile([P, TD, DQK], F32R)
                wv_bf = proj_sb.tile([P, TD, DV], BF16)

                with nc.named_scope("load"):
                    # W_k first (it gates the K projection -> broadcast) and
                    # per-tile contiguous DMAs.
                    for di in range(TD):
                        nc.sync.dma_start(
                            out=wk[:, di, :], in_=wk_d[di * P : (di + 1) * P, :]
                        )
                    for tj in range(TQ):
                        nc.sync.dma_start(
                            out=xa[:, tj, :], in_=x_d[tj * P : (tj + 1) * P, :]
                        )
                    for di in range(TD):
                        nc.sync.dma_start(
                            out=wv[:, di, :], in_=wv_d[di * P : (di + 1) * P, :]
                        )
                    for di in range(TD):
                        nc.sync.dma_start(
                            out=wq[:, di, :], in_=wq_d[di * P : (di + 1) * P, :]
                        )
                    nc.vector.tensor_copy(out=wk_r, in_=wk)

                # x^T (PE transposes), evacuations split across DVE/ACT.
                for tg in range(2):
                    with nc.named_scope(f"xT_{tg}"):
                        for di in range(TD):
                            ps_xt = ps_xt_pool.tile([P, 4 * P], F32, tag="ps_xt")
                            for j in range(4):
                                tj = tg * 4 + j
                                nc.tensor.transpose(
                                    ps_xt[:, j * P : (j + 1) * P],
                                    xa[:, tj, di * P : (di + 1) * P],
                                    ident_f32,
                                )
                            sl = slice(tg * 4 * P, (tg + 1) * 4 * P)
                            if di % 2 == 0:
                                nc.vector.tensor_copy(
                                    out=xT_r[:, di, sl], in_=ps_xt
                                )
                            else:
                                nc.scalar.copy(out=xT_r[:, di, sl], in_=ps_xt)
                    with nc.named_scope(f"kT_proj_{tg}"):
                        ps_k = ps_mm_pool.tile([P, 512], F32, tag="ps_mm")
                        for di in range(TD):
                            nc.tensor.matmul(
                                ps_k,
                                wk_r[:, di, :],
                                xT_r[:, di, tg * 512 : (tg + 1) * 512],
                                start=(di == 0),
                                stop=(di == TD - 1),
                            )
                        nc.vector.tensor_copy(
                            out=kT_h[:, tg * 512 : (tg + 1) * 512], in_=ps_k
                        )

                # p2p K broadcast: slot d == relative tpb d (XOR routing).
                # Cross-core traffic is only safe after the NRT sync
                # barrier (T_B, ~68us); the earliest bass-visible proof is
                # the prelude AllGather's then_inc on the kernel-entry
                # barrier sem (~T_B+20us). Gate the triggers on it (preps
                # are descgen only, no traffic). Sending earlier loses
                # packets and hangs the receivers' arrival waits.
                bsem = nc._bir_kernel_barrier_sem
                binc = nc.bir_kernel_barrier_sem_inc
                with nc.named_scope("bc_k"):
                    for d in range(1, N_CORES):
                        rdests = [None] * N_CORES
                        rdests[d] = (0, d)
                        nc.gpsimd.remote_dma_broadcast(
                            out_ap=kT_h[:, d * NLOC : (d + 1) * NLOC],
                            in_ap=kT_h[:, 0:NLOC],
                            remote_sem=sem_k[d - 1],
                            local_sem=sem_spare,
                            rdests=rdests,
                        )
                    trig = nc.gpsimd.trigger_dma(count=None)
                    pend_waits.append((trig, bsem, binc))

                with nc.named_scope("q_proj"):
                    nc.vector.tensor_copy(out=wq_r, in_=wq)
                    for h in range(NLOC // 512):
                        ps_q = ps_mm_pool.tile([P, 512], F32, tag="ps_mm")
                        for di in range(TD):
                            nc.tensor.matmul(
                                ps_q,
                                wq_r[:, di, :],
                                xT_r[:, di, h * 512 : (h + 1) * 512],
                                start=(di == 0),
                                stop=(di == TD - 1),
                            )
                        nc.vector.tensor_scalar_mul(
                            qT_h[:, h * 512 : (h + 1) * 512], ps_q, SCALE
                        )

                with nc.named_scope("v_proj"):
                    nc.vector.tensor_copy(out=wv_bf, in_=wv)
                    for di in range(TD):
                        nc.scalar.copy(
                            out=xT_bf[:, di, :], in_=xT_r[:, di, :].bitcast(F32)
                        )
                    for tj in range(TQ):
                        ps_v = ps_v_pool.tile([P, DV], F32, tag="ps_v")
                        for di in range(TD):
                            nc.tensor.matmul(
                                ps_v,
                                xT_bf[:, di, tj * P : (tj + 1) * P],
                                wv_bf[:, di, :],
                                start=(di == 0),
                                stop=(di == TD - 1),
                            )
                        nc.vector.tensor_copy(out=vf[:, tj, :], in_=ps_v)

                with nc.named_scope("bc_v"):
                    for d in range(1, N_CORES):
                        rdests = [None] * N_CORES
                        rdests[d] = (0, d)
                        nc.gpsimd.remote_dma_broadcast(
                            out_ap=vf[:, d * TQ : (d + 1) * TQ, :],
                            in_ap=vf[:, 0:TQ, :],
                            remote_sem=sem_v[d - 1],
                            local_sem=sem_spare,
                            rdests=rdests,
                        )
                    trig = nc.gpsimd.trigger_dma(count=None)
                    pend_waits.append((trig, bsem, binc))

            # ---------------- attention ----------------
            with (
                tc.tile_pool(name="attn_sb", bufs=4) as attn_sb,
                tc.tile_pool(name="stat_sb", bufs=2) as stat_sb,
                tc.tile_pool(name="ps_st", bufs=4, space="PSUM") as ps_st_pool,
                tc.tile_pool(name="ps_stat", bufs=2, space="PSUM") as ps_stat_pool,
                tc.tile_pool(name="ps_od", bufs=1, space="PSUM") as ps_od_pool,
            ):
                # Multi-use PSUM bank: den accumulators on rows 0 (chunk 0)
                # and 32 (chunk 1); den-transpose scratch (only touched after
                # the den row has been read out).
                ps_misc = ps_od_pool.tile([P, QC], F32, tag="ps_misc", bufs=1)

                route_ctr = [0]

                def stats_unit(qt, ch, kwait):
                    """One stats chunk: fp16 matmul + max-reduce."""
                    ps_stat = ps_st_pool.tile([P, SCH], F32, tag="ps_st")
                    mm = nc.tensor.matmul(
                        ps_stat,
                        qT_h[:, qt * P : (qt + 1) * P],
                        kT_h[:, ch * SCH : (ch + 1) * SCH],
                        start=True,
                        stop=True,
                    )
                    if kwait is not None:
                        pend_waits.append((mm, kwait[0], kwait[1]))
                        if warm_last[0] is not None:
                            add_dep_helper(
                                mm.ins,
                                warm_last[0].ins,
                                reason="gated stats after warm bridge",
                            )
                    route_ctr[0] += 1
                    if route_ctr[0] % 3 == 0:
                        nc.vector.reduce_max(
                            mx_all[:, qt, ch : ch + 1],
                            ps_stat,
                            axis=mybir.AxisListType.X,
                        )
                    else:
                        sh = stat_sb.tile([P, SCH], F16, tag="stat_h", bufs=3)
                        nc.scalar.copy(out=sh, in_=ps_stat)
                        nc.vector.reduce_max(
                            mx_all[:, qt, ch : ch + 1],
                            sh,
                            axis=mybir.AxisListType.X,
                        )
                    return mm

                def stats_combine(qt):
                    """Combine chunk maxes -> -m_hat -> negm_row slice."""
                    m1 = stat_sb.tile([P, 1], F32, tag="m1")
                    negm = stat_sb.tile([P, 1], F32, tag="negm")
                    nc.vector.reduce_max(
                        m1, mx_all[:, qt, :], axis=mybir.AxisListType.X
                    )
                    nc.vector.tensor_scalar_mul(negm, m1, -1.0)
                    ps_nm = ps_stat_pool.tile([1, P], F32, tag="ps_stat")
                    nc.tensor.transpose(ps_nm, negm, ident_f32)
                    nc.vector.tensor_copy(
                        out=negm_row[0:1, qt * P : (qt + 1) * P], in_=ps_nm
                    )

                def warm_mm(col):
                    """Junk matmul into the (otherwise idle) ps_o bank:
                    keeps the PE HAM activity monitor armed across the
                    barrier-gated wait for the remote K shards, so the
                    remote-slot stats and main run at full clock."""
                    ps_w = ps_st_pool.tile([P, QC], F32, tag="ps_st")
                    return nc.tensor.matmul(
                        ps_w,
                        ident_h,
                        kT_h[:, col : col + QC],
                        start=True,
                        stop=True,
                    )

                # Stats, slot-major: local shard first, then remote shards
                # as they arrive (gated by the per-slot arrival semaphores).
                # The warm bridge covers local-work-done (~55us) to
                # shard-arrival (~my send trigger + transfer, ~95us).
                warm_last = [None]
                with nc.named_scope("stats"):
                    for d in range(N_CORES):
                        kwait = (sem_k[d - 1], 2) if d > 0 else None
                        if d == 1:
                            with nc.named_scope("warm"):
                                for w in range(155):
                                    warm_last[0] = warm_mm((w % 2) * QC)
                        for qt in range(TQ):
                            for j in range(2):
                                smm = stats_unit(qt, 2 * d + j, kwait)
                                if d > 0:
                                    ps_w = ps_od_pool.tile(
                                        [P, QC], F32, tag="ps_o", bufs=1
                                    )
                                    wmm = nc.tensor.matmul(
                                        ps_w,
                                        ident_h,
                                        kT_h[:, j * QC : (j + 1) * QC],
                                        start=True,
                                        stop=True,
                                    )
                                    add_dep_helper(
                                        wmm.ins,
                                        smm.ins,
                                        reason="clock-hold warm after stats",
                                    )
                            if d == N_CORES - 1:
                                stats_combine(qt)

                def shift_prologue(qc, ps_st):
                    """PSUM <- ident^T @ nb (full-activity shift)."""
                    nc.tensor.matmul(
                        ps_st, ident_h, nb[:, qc, :], start=True, stop=False
                    )

                def st_accum(qc, kv, ps_st):
                    qs = qc * QC
                    nc.tensor.matmul(
                        ps_st,
                        kT_h[:, kv * P : (kv + 1) * P],
                        qT_h[:, qs : qs + QC],
                        start=False,
                        stop=True,
                    )

                def out_phase(qc, ps_o, ps_den):
                    """Evacuate O^T + den for chunk qc: transpose, scale, DMA."""
                    qs = qc * QC
                    den_row = stat_sb.tile([1, QC], F32, tag="den_row")
                    nc.vector.tensor_copy(out=den_row, in_=ps_den)
                    ps_rd = ps_misc[:, 0 : QC // P]
                    for j in range(QC // P):
                        nc.tensor.transpose(
                            ps_rd[:, j : j + 1],
                            den_row[0:1, j * P : (j + 1) * P],
                            ones_f[0:1, 0:1],
                        )
                    # reciprocal AFTER transposing to [128, 4]: 128 DVE lanes
                    # instead of one (a [1,512] reciprocal costs 3.3us serial)
                    den_col = stat_sb.tile([P, QC // P], F32, tag="den_col")
                    nc.vector.tensor_copy(out=den_col, in_=ps_rd)
                    rden_col = stat_sb.tile([P, QC // P], F32, tag="rden_col")
                    nc.vector.reciprocal(rden_col, den_col)

                    oT_sb = stat_sb.tile([P, QC], F32, tag="oT_sb")
                    nc.vector.tensor_copy(out=oT_sb, in_=ps_o)
                    o_nat = stat_sb.tile([P, QC // P, DV], F32, tag="o_nat")
                    ps_on = ps_st_pool.tile([P, QC], F32, tag="ps_st")
                    for j in range(QC // P):
                        nc.tensor.transpose(
                            ps_on[:, j * P : (j + 1) * P],
                            oT_sb[:, j * P : (j + 1) * P],
                            ident_f32,
                        )
                    for j in range(QC // P):
                        nc.vector.tensor_scalar_mul(
                            o_nat[:, j, :],
                            ps_on[:, j * P : (j + 1) * P],
                            rden_col[:, j : j + 1],
                        )
                    nc.sync.dma_start(
                        out=out_d[qs : qs + QC, :].rearrange(
                            "(t p) d -> p t d", p=P
                        ),
                        in_=o_nat,
                    )

                # main chunks (stats complete before main_0)
                pending_out = []
                for qc in range(NQC):
                    with nc.named_scope(f"main_{qc}"):
                        nc.gpsimd.partition_broadcast(
                            nb[:, qc, :], negm_row[0:1, qc * QC : (qc + 1) * QC]
                        )
                        ps_o = ps_od_pool.tile([P, QC], F32, tag="ps_o", bufs=1)
                        ps_den = ps_misc[qc * 32 : qc * 32 + 1, :]
                        aT_tiles = {}
                        aTs_tiles = {}
                        aTq_tiles = {}
                        aTo_tiles = {}
                        ps_tiles = {}
                        for kv in range(NKV + PIPE):
                            if kv < NKV:
                                if kv % 2 == 0:
                                    # both shift prologues back-to-back: the
                                    # identity stationary loads only once
                                    ps_tiles[kv] = ps_st_pool.tile(
                                        [P, QC], F32, tag="ps_st", name="ps_a"
                                    )
                                    ps_tiles[kv + 1] = ps_st_pool.tile(
                                        [P, QC], F32, tag="ps_st", name="ps_b"
                                    )
                                    shift_prologue(qc, ps_tiles[kv])
                                    shift_prologue(qc, ps_tiles[kv + 1])
                                ps_st = ps_tiles.pop(kv)
                                st_accum(qc, kv, ps_st)
                                if qc == 1 and kv == 1 and pending_out:
                                    with nc.named_scope("out_0"):
                                        out_phase(*pending_out.pop())
                                aT = attn_sb.tile([P, QC], BF16, tag="aT", bufs=8)
                                nc.scalar.activation(
                                    aT, ps_st, mybir.ActivationFunctionType.Exp
                                )
                                aT_tiles[kv] = aT
                                if kv % 2 == 1:
                                    # pair-sum then quad-sum in bf16 (DVE 4x
                                    # mode) to quarter the den matmuls
                                    aTs = attn_sb.tile(
                                        [P, QC], BF16, tag="aTs", bufs=3
                                    )
                                    nc.vector.tensor_tensor(
                                        aTs,
                                        aT_tiles[kv - 1],
                                        aT_tiles[kv],
                                        mybir.AluOpType.add,
                                    )
                                    aTs_tiles[kv // 2] = aTs
                                if kv % 4 == 3:
                                    aTq = attn_sb.tile(
                                        [P, QC], BF16, tag="aTq", bufs=3
                                    )
                                    nc.vector.tensor_tensor(
                                        aTq,
                                        aTs_tiles.pop(kv // 2 - 1),
                                        aTs_tiles.pop(kv // 2),
                                        mybir.AluOpType.add,
                                    )
                                    aTq_tiles[kv // 4] = aTq
                                if kv % 8 == 7:
                                    aTo = attn_sb.tile(
                                        [P, QC], BF16, tag="aTo", bufs=3
                                    )
                                    nc.vector.tensor_tensor(
                                        aTo,
                                        aTq_tiles.pop(kv // 4 - 1),
                                        aTq_tiles.pop(kv // 4),
                                        mybir.AluOpType.add,
                                    )
                                    aTo_tiles[kv // 8] = aTo
                            k2 = kv - PIPE
                            if k2 >= 0:
                                av = nc.tensor.matmul(
                                    ps_o,
                                    vf[:, k2, :],
                                    aT_tiles[k2],
                                    start=(k2 == 0),
                                    stop=(k2 == NKV - 1),
                                )
                                slot = k2 // TQ
                                if slot > 0:
                                    pend_waits.append((av, sem_v[slot - 1], 2))
                                if k2 % 8 == 7:
                                    qr = k2 // 8
                                    nc.tensor.matmul(
                                        ps_den,
                                        ones_col_bf,
                                        aTo_tiles[qr],
                                        start=(qr == 0),
                                        stop=(qr == NKV // 8 - 1),
                                    )
                                    del aTo_tiles[qr]
                                del aT_tiles[k2]
                    if qc == 0:
                        pending_out.append((qc, ps_o, ps_den))
                    else:
                        with nc.named_scope(f"out_{qc}"):
                            out_phase(qc, ps_o, ps_den)

    # Attach the cross-core sem-ge waits now that scheduling is done.
    # The prelude AllGather (nothing waits on it) keeps one collective in
    # the NEFF so the runtime initializes cross-core comms.
    nc._bir_kernel_barrier_sem_replica_groups.extend(set(g) for g in groups)
    for bi, sem, val in pend_waits:
        # check=False: scheduler-assigned waits may already be present;
        # compile's generate_event_semaphores legalizes the wait count.
        bi.wait_op(sem, val, "sem-ge", check=False)

    nc.compile()
    return nc


_NC_CACHE = None


def _get_nc():
    global _NC_CACHE
    if _NC_CACHE is None:
        _NC_CACHE = build_nc()
    return _NC_CACHE


def run(inputs, trace=False, **kw):
    """Run the SPMD kernel; returns BassKernelResults."""
    nc = _get_nc()
    x = np.asarray(inputs["x"], dtype=np.float32)
    wq = np.asarray(inputs["W_q"], dtype=np.float32)
    wk = np.asarray(inputs["W_k"], dtype=np.float32)
    wv = np.asarray(inputs["W_v"], dtype=np.float32)
    in_maps = [
        {
            "x": np.ascontiguousarray(x[c * NLOC : (c + 1) * NLOC]),
            "W_q": wq,
            "W_k": wk,
            "W_v": wv,
        }
        for c in range(N_CORES)
    ]
    return run_bass_kernel_spmd(
        nc, in_maps, core_ids=list(range(N_CORES)), trace=trace, **kw
    )


def kernel(**inputs):
    res = run(inputs, trace=False)
    return np.concatenate([res.results[c]["out"] for c in range(N_CORES)], axis=0)
